# revision 1
# baseline (speedup 1.0000x reference)
"""Causal self-attention (B=2, T=2048, C=1024, H=16) on 8 TRN2 NeuronCores.

Sharding: tensor-parallel over heads (2 heads/core) for QKV projection and
attention; AllToAll converts the head-sharded attention output into a
sequence-sharded layout; each core then computes its 512-row slice of the
output projection. Host only slices/casts inputs and concatenates outputs.

Device math in bf16 with fp32 PSUM accumulation:
  - x is pre-transposed on host to xT [C, B*T] (bf16) so every matmul
    contraction has channels on the partition axis.
  - Scores are built transposed (S^T [keys, queries]) so softmax
    normalization sums arrive for free from a ones-augmented P^T @ [V|1]
    matmul, and no on-device transposes are needed anywhere.
  - exp on ScalarE (fp32-accurate LUT); no max-subtraction needed since
    scores are O(+-8).
"""
import os
import math
import threading

import numpy as np
import ml_dtypes

import concourse.bass as bass
import concourse.tile as tile
from concourse import mybir, bacc, bass_utils

B, T, C, H = 2, 2048, 1024, 16
D = C // H                 # 64
NCORES = 8
HPC = H // NCORES          # heads per core = 2
HC = HPC * D               # head-channels per core = 128
BT = B * T                 # 4096
TQ = 512                   # query chunk
TKT = 128                  # key tile
ROWS = BT // NCORES        # output rows per core = 512
SM_SCALE = 1.0 / math.sqrt(D)

F32 = mybir.dt.float32
BF16 = mybir.dt.bfloat16
BF16_NP = ml_dtypes.bfloat16


def _build_program():
    nc = bacc.Bacc("TRN2", target_bir_lowering=False, debug=False,
                   num_devices=NCORES)
    xt = nc.dram_tensor("xt", [C, BT], BF16, kind="ExternalInput").ap()
    wqkv = nc.dram_tensor("wqkv", [C, 3 * HC], BF16, kind="ExternalInput").ap()
    wproj = nc.dram_tensor("wproj", [C, C], BF16, kind="ExternalInput").ap()
    bq = nc.dram_tensor("bq", [HC, 1], F32, kind="ExternalInput").ap()
    bk = nc.dram_tensor("bk", [HC, 1], F32, kind="ExternalInput").ap()
    bv = nc.dram_tensor("bv", [1, HC], BF16, kind="ExternalInput").ap()
    bproj = nc.dram_tensor("bproj", [1, C], BF16, kind="ExternalInput").ap()
    masks = nc.dram_tensor("masks", [TQ // TKT, TKT, TQ], BF16,
                           kind="ExternalInput").ap()
    outp = nc.dram_tensor("out", [ROWS, C], F32, kind="ExternalOutput").ap()

    KT = C // 128          # 8 contraction tiles over channels
    NCH = BT // TQ         # 8 T-chunks over B*T
    SPC = TQ // D          # 8 strips of 64 rows per chunk (one per core)

    with tile.TileContext(nc) as tc:
        with (
            tc.tile_pool(name="consts", bufs=1) as consts,
            tc.tile_pool(name="xpool", bufs=2) as xpool,
            tc.tile_pool(name="ppool", bufs=6) as ppool,
            tc.tile_pool(name="npool", bufs=2) as npool,
            tc.tile_pool(name="opool", bufs=2) as opool,
            tc.tile_pool(name="ps_o", bufs=2, space="PSUM") as ps_o,
            tc.tile_pool(name="dram", bufs=1, space="DRAM") as dram,
        ):
            # ---- stage 0: weights & constants ----
            wqkv_sb = []
            for kt in range(KT):
                w1 = consts.tile([128, 3 * HC], BF16, name=f"wqkv_sb{kt}")
                nc.sync.dma_start(out=w1, in_=wqkv[128 * kt:128 * (kt + 1), :])
                wqkv_sb.append(w1)
            # big weights not needed until ~100us in: keep them off the SP
            # HWDGE queue so the stage-1 stream starts immediately
            wproj_sb = []
            for kt in range(KT):
                w2 = consts.tile([128, C], BF16, name=f"wproj_sb{kt}")
                nc.gpsimd.dma_start(out=w2, in_=wproj[128 * kt:128 * (kt + 1), :])
                wproj_sb.append(w2)
            ones_sb = consts.tile([1, 128], BF16, name="ones_sb")
            nc.vector.memset(ones_sb, 1.0)
            bq_sb = consts.tile([HC, 1], F32, name="bq_sb")
            nc.sync.dma_start(out=bq_sb, in_=bq)
            bk_sb = consts.tile([HC, 1], F32, name="bk_sb")
            nc.sync.dma_start(out=bk_sb, in_=bk)
            bv_sb = consts.tile([1, HC], BF16, name="bv_sb")
            nc.sync.dma_start(out=bv_sb, in_=bv)
            bproj_sb = consts.tile([1, C], BF16, name="bproj_sb")
            nc.sync.dma_start(out=bproj_sb, in_=bproj)
            masks_sb = consts.tile([TKT, TQ // TKT, TQ], BF16, name="masks_sb")
            nc.gpsimd.dma_start(out=masks_sb, in_=masks.rearrange("r p q -> p r q"))

            qT_b = [consts.tile([HC, T], BF16, name=f"qT_sb{b}")
                    for b in range(B)]
            kT_b = [consts.tile([HC, T], BF16, name=f"kT_sb{b}")
                    for b in range(B)]
            v_sb = [consts.tile([128, 2 * (D + 1)], BF16, name=f"v_sb{tt}")
                    for tt in range(BT // 128)]

            # per-chunk exchange buffers: block s of chunk c = queries
            # [64s, 64s+64) of that chunk, owned by core s
            a2a_in = [dram.tile([NCORES, HC, D], BF16, name=f"a2a_in{c}")
                      for c in range(NCH)]
            a2a_out = [dram.tile([NCORES, HC, D], BF16, name=f"a2a_out{c}")
                       for c in range(NCH)]

            def stage4_pair(cA, cB):
                """Output projection for two 64-row strips, column-packed."""
                ylhs = {}
                for ci, c in enumerate((cA, cB)):
                    yy = opool.tile([128, SPC, D], BF16, tag=f"ylhs{ci}",
                                    name=f"ylhs{ci}")
                    nc.sync.dma_start(
                        out=yy, in_=a2a_out[c].rearrange("k p q -> p k q"))
                    ylhs[c] = yy
                for n in range(C // TQ):
                    po = ps_o.tile([128, TQ], F32, tag="po", name="po")
                    for ci, c in enumerate((cA, cB)):
                        pslice = po[D * ci:D * (ci + 1), :]
                        for kt in range(KT):
                            nc.tensor.matmul(
                                pslice,
                                lhsT=ylhs[c][:, kt, :],
                                rhs=wproj_sb[kt][:, TQ * n:TQ * (n + 1)],
                                start=(kt == 0), stop=False)
                        nc.tensor.matmul(
                            pslice, lhsT=ones_sb[:, 0:D],
                            rhs=bproj_sb[:, TQ * n:TQ * (n + 1)],
                            start=False, stop=True)
                    osb = opool.tile([128, TQ], F32, tag="osb", name="osb")
                    nc.vector.tensor_copy(out=osb, in_=po)
                    for ci, c in enumerate((cA, cB)):
                        nc.sync.dma_start(
                            out=outp[D * c:D * (c + 1), TQ * n:TQ * (n + 1)],
                            in_=osb[D * ci:D * (ci + 1), :])

            done_chunks = []
            for b in range(B):
                # ---- stage 1: QKV projection for this batch ----
                with (
                    tc.tile_pool(name=f"ps_qk{b}", bufs=3, space="PSUM") as ps_qk,
                    tc.tile_pool(name=f"ps_v{b}", bufs=2, space="PSUM") as ps_v,
                ):
                    for cl in range(NCH // B):
                        c = (NCH // B) * b + cl
                        xt_t = []
                        for kt in range(KT):
                            xx = xpool.tile([128, TQ], BF16, tag=f"xt{kt}")
                            nc.sync.dma_start(
                                out=xx,
                                in_=xt[128 * kt:128 * (kt + 1),
                                       TQ * c:TQ * (c + 1)])
                            xt_t.append(xx)
                        for which, off, bias, scale in (
                            ("q", 0, bq_sb, 1.0),
                            ("k", HC, bk_sb, SM_SCALE),
                        ):
                            ps = ps_qk.tile([HC, TQ], F32, tag="qk")
                            for kt in range(KT):
                                nc.tensor.matmul(
                                    ps,
                                    lhsT=wqkv_sb[kt][:, off:off + HC],
                                    rhs=xt_t[kt],
                                    start=(kt == 0), stop=(kt == KT - 1))
                            dst = qT_b[b] if which == "q" else kT_b[b]
                            nc.scalar.activation(
                                out=dst[:, TQ * cl:TQ * (cl + 1)], in_=ps,
                                func=mybir.ActivationFunctionType.Identity,
                                bias=bias, scale=scale)
                        # V (natural layout, ones-augmented)
                        for s in range(TQ // 128):
                            tt = 4 * c + s
                            ps = ps_v.tile([128, HC], F32, tag="v")
                            for kt in range(KT):
                                nc.tensor.matmul(
                                    ps,
                                    lhsT=xt_t[kt][:, 128 * s:128 * (s + 1)],
                                    rhs=wqkv_sb[kt][:, 2 * HC:3 * HC],
                                    start=(kt == 0), stop=False)
                            nc.tensor.matmul(ps, lhsT=ones_sb, rhs=bv_sb,
                                             start=False, stop=True)
                            vt = v_sb[tt]
                            nc.vector.tensor_copy(out=vt[:, 0:D], in_=ps[:, 0:D])
                            nc.vector.tensor_copy(out=vt[:, D + 1:2 * D + 1],
                                                  in_=ps[:, D:2 * D])
                            nc.vector.memset(vt[:, D:D + 1], 1.0)
                            nc.vector.memset(vt[:, 2 * D + 1:2 * D + 2], 1.0)

                # ---- stage 2: attention for this batch, largest chunks
                # first; each chunk's exchange + output projection follows
                # immediately and hides under later chunks' attention ----
                with (
                    tc.tile_pool(name=f"ps_s{b}", bufs=4, space="PSUM") as ps_s,
                    tc.tile_pool(name=f"ps_y{b}", bufs=1, space="PSUM") as ps_y,
                ):
                    for jl in reversed(range(T // TQ)):
                        cidx = (T // TQ) * b + jl
                        q0 = TQ * jl
                        nkt = (TQ // TKT) * (jl + 1)
                        y_ps = [ps_y.tile([D + 1, TQ], F32, tag=f"y{h}",
                                          name=f"y_ps{h}")
                                for h in range(HPC)]
                        pts = []
                        for kt in range(nkt):
                            k0 = TKT * kt
                            r = kt - (TQ // TKT) * jl
                            pt_pair = []
                            for h in range(HPC):
                                hp = D * h
                                ss = ps_s.tile([TKT, TQ], F32, tag="s",
                                               name=f"ss{h}")
                                nc.tensor.matmul(
                                    ss,
                                    lhsT=kT_b[b][hp:hp + D, k0:k0 + TKT],
                                    rhs=qT_b[b][hp:hp + D, q0:q0 + TQ],
                                    start=True, stop=True)
                                pt = ppool.tile([TKT, TQ], BF16, tag=f"pt{h}",
                                                name=f"pt{h}")
                                nc.scalar.activation(
                                    out=pt, in_=ss,
                                    func=mybir.ActivationFunctionType.Exp)
                                if r >= 0:
                                    nc.vector.tensor_mul(pt, pt,
                                                         masks_sb[:, r, :])
                                pt_pair.append(pt)
                            pts.append(pt_pair)
                        for kt in range(nkt):
                            vt = v_sb[(T // 128) * b + kt]
                            for h in range(HPC):
                                nc.tensor.matmul(
                                    y_ps[h],
                                    lhsT=vt[:, (D + 1) * h:(D + 1) * (h + 1)],
                                    rhs=pts[kt][h],
                                    start=(kt == 0), stop=(kt == nkt - 1))
                        for h in range(HPC):
                            recip = npool.tile([1, TQ], F32, tag="recip")
                            nc.vector.reciprocal(recip, y_ps[h][D:D + 1, :])
                            recip_b = npool.tile([D, TQ], F32, tag="recipb")
                            nc.gpsimd.partition_broadcast(recip_b, recip)
                            yt = npool.tile([D, TQ], BF16, tag="yt")
                            nc.vector.tensor_mul(yt, y_ps[h][0:D, :], recip_b)
                            nc.sync.dma_start(
                                out=a2a_in[cidx][:, D * h:D * (h + 1), :]
                                    .rearrange("s p q -> p s q"),
                                in_=yt.rearrange("p (s q) -> p s q", s=SPC))
                        nc.gpsimd.collective_compute(
                            "AllToAll", mybir.AluOpType.bypass,
                            replica_groups=[list(range(NCORES))],
                            ins=[a2a_in[cidx].opt()],
                            outs=[a2a_out[cidx].opt()])
                        done_chunks.append(cidx)
                        if len(done_chunks) % 2 == 0:
                            stage4_pair(done_chunks[-2], done_chunks[-1])

    nc.compile()
    return nc


_lock = threading.Lock()
_cached_nc = None
last_results = None  # BassKernelResults of the most recent kernel() call


def _get_program():
    global _cached_nc
    with _lock:
        if _cached_nc is None:
            _cached_nc = _build_program()
    return _cached_nc


def _host_inputs(x, W_qkv, b_qkv, W_proj, b_proj):
    bf = lambda a: np.ascontiguousarray(a).astype(BF16_NP)
    x = np.asarray(x, dtype=np.float32)
    W_qkv = np.asarray(W_qkv, dtype=np.float32)
    b_qkv = np.asarray(b_qkv, dtype=np.float32)
    W_proj = np.asarray(W_proj, dtype=np.float32)
    b_proj = np.asarray(b_proj, dtype=np.float32)

    xt = bf(x.reshape(BT, C).T)                     # [C, BT]
    wproj = bf(W_proj)                              # [C, C]
    bproj = bf(b_proj.reshape(1, C))
    r = np.arange(TQ // TKT)[:, None, None]
    k = np.arange(TKT)[None, :, None]
    q = np.arange(TQ)[None, None, :]
    masks = ((k + TKT * r) <= q).astype(BF16_NP)    # [4, 128, 512]

    in_maps = []
    for i in range(NCORES):
        sel = slice(HC * i, HC * (i + 1))
        wq = W_qkv[:, sel]
        wk = W_qkv[:, C + HC * i:C + HC * (i + 1)]
        wv = W_qkv[:, 2 * C + HC * i:2 * C + HC * (i + 1)]
        in_maps.append({
            "xt": xt,
            "wqkv": bf(np.concatenate([wq, wk, wv], axis=1)),
            "wproj": wproj,
            "bq": np.ascontiguousarray(
                b_qkv[sel].reshape(HC, 1)).astype(np.float32),
            "bk": np.ascontiguousarray(
                (b_qkv[C + HC * i:C + HC * (i + 1)] * SM_SCALE)
                .reshape(HC, 1)).astype(np.float32),
            "bv": b_qkv[2 * C + HC * i:2 * C + HC * (i + 1)]
                .reshape(1, HC).astype(BF16_NP),
            "bproj": bproj,
            "masks": masks,
        })
    return in_maps


def kernel(x, W_qkv, b_qkv, W_proj, b_proj):
    global last_results
    nc = _get_program()
    in_maps = _host_inputs(x, W_qkv, b_qkv, W_proj, b_proj)
    trace = bool(int(os.environ.get("KERNEL_TRACE", "0")))
    res = bass_utils.run_bass_kernel_spmd(
        nc, in_maps, core_ids=list(range(NCORES)), trace=trace)
    last_results = res
    # core s's output rows are strip s (64 rows) of every 512-row chunk
    arr = np.stack([res.results[s]["out"].reshape(BT // TQ, D, C)
                    for s in range(NCORES)], axis=1)   # [chunk, core, 64, C]
    return np.ascontiguousarray(arr.reshape(B, T, C))



# revision 10
# speedup vs baseline: 1.3260x; 1.3260x over previous
"""Causal self-attention (B=2, T=2048, C=1024, H=16) on 8 TRN2 NeuronCores.

Sharding: tensor-parallel over heads (2 heads/core) for QKV projection and
attention; AllToAll converts the head-sharded attention output into a
sequence-sharded layout; each core then computes its 512-row slice of the
output projection. Host only slices/casts inputs and concatenates outputs.

v2 schedule (vs. the 345us baseline):
  - One dense QKV phase for both batches keeps the PE HAM-warm from the start.
  - Scores for the two heads are row-packed (K=64 each at base partitions
    0/64) so the pair runs concurrently in the PE array.
  - exp is batched over head-pairs ([128, 2, 512] PSUM -> one ACT call) and
    restricted to the un-masked column range on diagonal tiles; fully-masked
    columns are memset instead of exp'd+masked, and the causal mask multiply
    shrinks to a single 128x128 tril block per head.
  - Softmax normalization: reciprocal_approx_fast on the ones-row, partition
    broadcast via a K=1 matmul (ones^T @ recip) into PSUM, one fused DVE
    multiply; the whole tail (broadcast, multiply, a2a DMA, collective) is
    deferred into the next chunk so the PE never waits on it.
  - Output projection packs two 64-row chunk strips into M=128 matmuls and
    runs at the end, hidden under the trailing AllToAlls; output is bf16.
"""
import os
import math
import threading

import numpy as np
import ml_dtypes

import concourse.bass as bass
import concourse.tile as tile
from concourse import mybir, bacc, bass_utils

B, T, C, H = 2, 2048, 1024, 16
D = C // H                 # 64
NCORES = 8
HPC = H // NCORES          # heads per core = 2
HC = HPC * D               # head-channels per core = 128
BT = B * T                 # 4096
TQ = 512                   # query chunk
TKT = 128                  # key tile
NCH = BT // TQ             # 8 chunks over B*T
SPC = TQ // D              # 8 strips of 64 rows per chunk (one per core)
ROWS = BT // NCORES        # output rows per core = 512
KT = C // 128              # 8 contraction tiles over channels
SM_SCALE = 1.0 / math.sqrt(D)

F32 = mybir.dt.float32
BF16 = mybir.dt.bfloat16
BF16_NP = ml_dtypes.bfloat16
DEBUG = bool(int(os.environ.get("KERNEL_DEBUG", "0")))


def _build_program():
    nc = bacc.Bacc("TRN2", target_bir_lowering=False, debug=False,
                   num_devices=NCORES)
    xt = nc.dram_tensor("xt", [C, BT], BF16, kind="ExternalInput").ap()
    wqkv = nc.dram_tensor("wqkv", [C, 3 * HC], BF16, kind="ExternalInput").ap()
    wproj = nc.dram_tensor("wproj", [C, C], BF16, kind="ExternalInput").ap()
    bq = nc.dram_tensor("bq", [HC, 1], F32, kind="ExternalInput").ap()
    bk = nc.dram_tensor("bk", [HC, 1], F32, kind="ExternalInput").ap()
    bv = nc.dram_tensor("bv", [1, HC], BF16, kind="ExternalInput").ap()
    bproj = nc.dram_tensor("bproj", [1, C], BF16, kind="ExternalInput").ap()
    maskt = nc.dram_tensor("maskt", [TKT, TKT], BF16, kind="ExternalInput").ap()
    outp = nc.dram_tensor("out", [ROWS, C], BF16, kind="ExternalOutput").ap()
    if DEBUG:
        dbg_qT = nc.dram_tensor("dbg_qT", [HC, T], BF16,
                                kind="ExternalOutput").ap()
        dbg_kT = nc.dram_tensor("dbg_kT", [HC, T], BF16,
                                kind="ExternalOutput").ap()
        dbg_v0 = nc.dram_tensor("dbg_v0", [128, HPC, D + 1], BF16,
                                kind="ExternalOutput").ap()
        dbg_pt = nc.dram_tensor("dbg_pt", [TKT, HPC, TQ], BF16,
                                kind="ExternalOutput").ap()
        dbg_yc = nc.dram_tensor("dbg_yc", [D, HPC, TQ], BF16,
                                kind="ExternalOutput").ap()
        dbg_rec = nc.dram_tensor("dbg_rec", [1, HPC, TQ], BF16,
                                 kind="ExternalOutput").ap()
        dbg_yt = nc.dram_tensor("dbg_yt", [D, HPC, TQ], BF16,
                                kind="ExternalOutput").ap()

    with tile.TileContext(nc) as tc:
        with (
            tc.tile_pool(name="consts", bufs=1) as consts,
            tc.tile_pool(name="xpool", bufs=2) as xpool,
            tc.tile_pool(name="ppool", bufs=3) as ppool,
            tc.tile_pool(name="npool", bufs=2) as npool,
            tc.tile_pool(name="ytpool", bufs=2) as ytpool,
            tc.tile_pool(name="opool", bufs=2) as opool,
            tc.tile_pool(name="dram", bufs=1, space="DRAM") as dram,
        ):
            # ---- stage 0: weights & constants ----
            wqkv_sb = []
            for kt in range(KT):
                w1 = consts.tile([128, 3 * HC], BF16, name=f"wqkv_sb{kt}")
                nc.sync.dma_start(out=w1, in_=wqkv[128 * kt:128 * (kt + 1), :])
                wqkv_sb.append(w1)
            # big weights not needed until late: keep them off the SP HWDGE
            # queue so the phase-1 stream starts immediately
            wproj_sb = []
            for kt in range(KT):
                w2 = consts.tile([128, C], BF16, name=f"wproj_sb{kt}")
                nc.gpsimd.dma_start(out=w2, in_=wproj[128 * kt:128 * (kt + 1), :])
                wproj_sb.append(w2)
            ones_bf = consts.tile([1, 128], BF16, name="ones_bf")
            nc.vector.memset(ones_bf, 1.0)
            bq_sb = consts.tile([HC, 1], F32, name="bq_sb")
            nc.sync.dma_start(out=bq_sb, in_=bq)
            bk_sb = consts.tile([HC, 1], F32, name="bk_sb")
            nc.sync.dma_start(out=bk_sb, in_=bk)
            bv_sb = consts.tile([1, HC], BF16, name="bv_sb")
            nc.sync.dma_start(out=bv_sb, in_=bv)
            bproj_sb = consts.tile([1, C], BF16, name="bproj_sb")
            nc.sync.dma_start(out=bproj_sb, in_=bproj)
            mask_sb = consts.tile([TKT, TKT], BF16, name="mask_sb")
            nc.sync.dma_start(out=mask_sb, in_=maskt)

            qT_b = [consts.tile([HC, T], BF16, name=f"qT_sb{b}")
                    for b in range(B)]
            kT_b = [consts.tile([HC, T], BF16, name=f"kT_sb{b}")
                    for b in range(B)]
            v_sb = [consts.tile([128, HPC, D + 1], BF16, name=f"v_sb{tt}")
                    for tt in range(BT // 128)]
            for vt in v_sb:
                nc.vector.memset(vt[:, :, D:D + 1], 1.0)

            # force the exp table-set load during phase 1 (ACT is idle there)
            dummy = consts.tile([1, 1], F32, name="dummy_exp")
            nc.scalar.activation(out=dummy, in_=ones_bf[0:1, 0:1],
                                 func=mybir.ActivationFunctionType.Exp)

            # per-chunk exchange buffers: block s of chunk c = queries
            # [64s, 64s+64) of that chunk, owned by core s
            a2a_in = [dram.tile([NCORES, HC, D], BF16, name=f"a2a_in{c}")
                      for c in range(NCH)]
            a2a_out = [dram.tile([NCORES, HC, D], BF16, name=f"a2a_out{c}")
                       for c in range(NCH)]

            # ---- phase 1: QKV projection, both batches, dense on PE ----
            with (
                tc.tile_pool(name="ps_qk", bufs=2, space="PSUM") as ps_qk,
                tc.tile_pool(name="ps_v", bufs=2, space="PSUM") as ps_v,
            ):
                for c in range(NCH):
                    b, jl = c // (NCH // B), c % (NCH // B)
                    xt_t = []
                    for kt in range(KT):
                        xx = xpool.tile([128, TQ], BF16, tag=f"xt{kt}")
                        nc.sync.dma_start(
                            out=xx,
                            in_=xt[128 * kt:128 * (kt + 1),
                                   TQ * c:TQ * (c + 1)])
                        xt_t.append(xx)
                    psq = ps_qk.tile([HC, TQ], F32, tag="q")
                    for kt in range(KT):
                        nc.tensor.matmul(psq, lhsT=wqkv_sb[kt][:, 0:HC],
                                         rhs=xt_t[kt],
                                         start=(kt == 0), stop=(kt == KT - 1))
                    nc.vector.tensor_scalar(
                        out=qT_b[b][:, TQ * jl:TQ * (jl + 1)], in0=psq,
                        scalar1=bq_sb, scalar2=None, op0=mybir.AluOpType.add)
                    psk = ps_qk.tile([HC, TQ], F32, tag="k")
                    for kt in range(KT):
                        nc.tensor.matmul(psk, lhsT=wqkv_sb[kt][:, HC:2 * HC],
                                         rhs=xt_t[kt],
                                         start=(kt == 0), stop=(kt == KT - 1))
                    nc.vector.tensor_scalar(
                        out=kT_b[b][:, TQ * jl:TQ * (jl + 1)], in0=psk,
                        scalar1=SM_SCALE, scalar2=bk_sb,
                        op0=mybir.AluOpType.mult, op1=mybir.AluOpType.add)
                    for s in range(TQ // 128):
                        tt = 4 * c + s
                        psv = ps_v.tile([128, HPC, D], F32, tag="v")
                        for kt in range(KT):
                            nc.tensor.matmul(
                                psv,
                                lhsT=xt_t[kt][:, 128 * s:128 * (s + 1)],
                                rhs=wqkv_sb[kt][:, 2 * HC:3 * HC],
                                start=(kt == 0), stop=False)
                        nc.tensor.matmul(psv, lhsT=ones_bf, rhs=bv_sb,
                                         start=False, stop=True)
                        nc.vector.tensor_copy(out=v_sb[tt][:, :, 0:D], in_=psv)

            if DEBUG:
                nc.sync.dma_start(out=dbg_qT, in_=qT_b[0])
                nc.sync.dma_start(out=dbg_kT, in_=kT_b[0])
                nc.sync.dma_start(out=dbg_v0, in_=v_sb[0])

            # ---- phase 2: attention, largest chunks first, batches
            # alternating; each chunk's normalization + exchange is deferred
            # into the next chunk so PE/ACT never wait on it ----
            order = [(b, jl) for jl in reversed(range(NCH // B))
                     for b in range(B)]
            with (
                tc.tile_pool(name="ps_s", bufs=2, space="PSUM") as ps_s,
                tc.tile_pool(name="ps_y", bufs=1, space="PSUM") as ps_y,
                tc.tile_pool(name="ps_r", bufs=1, space="PSUM") as ps_r,
            ):
                pending = None
                for (b, jl) in order:
                    cidx = (NCH // B) * b + jl
                    nkt = (TQ // TKT) * (jl + 1)
                    q0 = TQ * jl
                    y = ps_y.tile([D + 1, HPC, TQ], F32, tag="y")
                    for kt in range(nkt):
                        r = kt - (TQ // TKT) * jl
                        k0 = TKT * kt
                        ss = ps_s.tile([TKT, HPC, TQ], F32, tag="s")
                        for h in range(HPC):
                            hp = D * h
                            nc.tensor.matmul(
                                ss[:, h, :],
                                lhsT=kT_b[b][hp:hp + D, k0:k0 + TKT],
                                rhs=qT_b[b][hp:hp + D, q0:q0 + TQ],
                                start=True, stop=True)
                        if kt == 0 and pending is not None:
                            pending()
                            pending = None
                        pt = ppool.tile([TKT, HPC, TQ], BF16, tag="pt")
                        c0 = TKT * r if r > 0 else 0
                        if c0 > 0:
                            nc.vector.memset(pt[:, :, 0:c0], 0.0)
                        nc.scalar.activation(
                            out=pt[:, :, c0:TQ], in_=ss[:, :, c0:TQ],
                            func=mybir.ActivationFunctionType.Exp)
                        if r >= 0:
                            m0 = TKT * r
                            for h in range(HPC):
                                nc.vector.tensor_mul(
                                    pt[:, h, m0:m0 + TKT],
                                    pt[:, h, m0:m0 + TKT], mask_sb)
                        if DEBUG and (b, jl) == (0, 3) and kt == 0:
                            nc.sync.dma_start(out=dbg_pt, in_=pt)
                        vt = v_sb[(T // 128) * b + kt]
                        for h in range(HPC):
                            nc.tensor.matmul(
                                y[:, h, :], lhsT=vt[:, h, :], rhs=pt[:, h, :],
                                start=(kt == 0), stop=(kt == nkt - 1))
                    # normalization front half (DVE only; y is complete here;
                    # the yc copy also frees the y banks for the next chunk).
                    # reciprocal_approx_fast (custom DVE op) cannot read the
                    # PSUM row at partition 64 -> stage it to SBUF partition 0.
                    den = npool.tile([1, HPC, TQ], F32, tag="den")
                    nc.vector.tensor_copy(out=den, in_=y[D:D + 1, :, :])
                    recf = npool.tile([1, HPC, TQ], F32, tag="recf")
                    nc.vector.reciprocal_approx_fast(out=recf, in_=den)
                    recb = npool.tile([1, HPC, TQ], BF16, tag="recb")
                    nc.vector.tensor_copy(out=recb, in_=recf)
                    yc = ytpool.tile([D, HPC, TQ], BF16, tag="yc")
                    nc.vector.tensor_copy(out=yc, in_=y[0:D, :, :])
                    if DEBUG and (b, jl) == (0, 3):
                        nc.sync.dma_start(out=dbg_yc, in_=yc)
                        nc.sync.dma_start(out=dbg_rec, in_=recb)

                    def make_tail(yc=yc, recb=recb, cidx=cidx):
                        def tail():
                            rbc = ps_r.tile([D, HPC, TQ], F32, tag="r")
                            for h in range(HPC):
                                nc.tensor.matmul(
                                    rbc[:, h, :], lhsT=ones_bf[0:1, 0:D],
                                    rhs=recb[0:1, h, :],
                                    start=True, stop=True)
                            yt = ytpool.tile([D, HPC, TQ], BF16, tag="yt")
                            nc.vector.tensor_mul(yt, yc, rbc)
                            if DEBUG and cidx == 3:
                                nc.sync.dma_start(out=dbg_yt, in_=yt)
                            for h in range(HPC):
                                nc.sync.dma_start(
                                    out=a2a_in[cidx][:, D * h:D * (h + 1), :]
                                        .rearrange("s p q -> p s q"),
                                    in_=yt[:, h, :]
                                        .rearrange("p (s q) -> p s q", s=SPC))
                            nc.gpsimd.collective_compute(
                                "AllToAll", mybir.AluOpType.bypass,
                                replica_groups=[list(range(NCORES))],
                                ins=[a2a_in[cidx].opt()],
                                outs=[a2a_out[cidx].opt()])
                        return tail
                    pending = make_tail()
                pending()
                pending = None

            # ---- phase 3: output projection, two 64-row strips packed into
            # M=128; pair order matches chunk completion order ----
            seq = [(NCH // B) * b + jl for (b, jl) in order]
            with tc.tile_pool(name="ps_o", bufs=2, space="PSUM") as ps_o:
                for i in range(0, NCH, 2):
                    cA, cB = seq[i], seq[i + 1]
                    yy = opool.tile([128, KT, 2, D], BF16, tag="yy")
                    for ci, cc in enumerate((cA, cB)):
                        nc.sync.dma_start(
                            out=yy[:, :, ci, :],
                            in_=a2a_out[cc].rearrange("k p q -> p k q"))
                    for n in range(C // TQ):
                        po = ps_o.tile([128, TQ], F32, tag="po")
                        for kt in range(KT):
                            nc.tensor.matmul(
                                po, lhsT=yy[:, kt, :, :],
                                rhs=wproj_sb[kt][:, TQ * n:TQ * (n + 1)],
                                start=(kt == 0), stop=False)
                        nc.tensor.matmul(
                            po, lhsT=ones_bf[0:1, 0:128],
                            rhs=bproj_sb[0:1, TQ * n:TQ * (n + 1)],
                            start=False, stop=True)
                        osb = opool.tile([128, TQ], BF16, tag="osb")
                        nc.scalar.activation(
                            out=osb, in_=po,
                            func=mybir.ActivationFunctionType.Copy)
                        for ci, cc in enumerate((cA, cB)):
                            nc.sync.dma_start(
                                out=outp[D * cc:D * (cc + 1),
                                         TQ * n:TQ * (n + 1)],
                                in_=osb[D * ci:D * (ci + 1), :])

    nc.compile()
    return nc


_lock = threading.Lock()
_cached_nc = None
last_results = None  # BassKernelResults of the most recent kernel() call


def _get_program():
    global _cached_nc
    with _lock:
        if _cached_nc is None:
            _cached_nc = _build_program()
    return _cached_nc


def _host_inputs(x, W_qkv, b_qkv, W_proj, b_proj):
    bf = lambda a: np.ascontiguousarray(a).astype(BF16_NP)
    x = np.asarray(x, dtype=np.float32)
    W_qkv = np.asarray(W_qkv, dtype=np.float32)
    b_qkv = np.asarray(b_qkv, dtype=np.float32)
    W_proj = np.asarray(W_proj, dtype=np.float32)
    b_proj = np.asarray(b_proj, dtype=np.float32)

    xt = bf(x.reshape(BT, C).T)                     # [C, BT]
    wproj = bf(W_proj)                              # [C, C]
    bproj = bf(b_proj.reshape(1, C))
    k = np.arange(TKT)[:, None]
    q = np.arange(TKT)[None, :]
    maskt = (k <= q).astype(BF16_NP)                # [128, 128] tril boundary

    in_maps = []
    for i in range(NCORES):
        sel = slice(HC * i, HC * (i + 1))
        wq = W_qkv[:, sel]
        wk = W_qkv[:, C + HC * i:C + HC * (i + 1)]
        wv = W_qkv[:, 2 * C + HC * i:2 * C + HC * (i + 1)]
        in_maps.append({
            "xt": xt,
            "wqkv": bf(np.concatenate([wq, wk, wv], axis=1)),
            "wproj": wproj,
            "bq": np.ascontiguousarray(
                b_qkv[sel].reshape(HC, 1)).astype(np.float32),
            "bk": np.ascontiguousarray(
                (b_qkv[C + HC * i:C + HC * (i + 1)] * SM_SCALE)
                .reshape(HC, 1)).astype(np.float32),
            "bv": b_qkv[2 * C + HC * i:2 * C + HC * (i + 1)]
                .reshape(1, HC).astype(BF16_NP),
            "bproj": bproj,
            "maskt": maskt,
        })
    return in_maps


def kernel(x, W_qkv, b_qkv, W_proj, b_proj):
    global last_results
    nc = _get_program()
    in_maps = _host_inputs(x, W_qkv, b_qkv, W_proj, b_proj)
    trace = bool(int(os.environ.get("KERNEL_TRACE", "0")))
    res = bass_utils.run_bass_kernel_spmd(
        nc, in_maps, core_ids=list(range(NCORES)), trace=trace)
    last_results = res
    # core s's output rows are strip s (64 rows) of every 512-row chunk
    arr = np.stack([np.asarray(res.results[s]["out"])
                    .astype(np.float32).reshape(BT // TQ, D, C)
                    for s in range(NCORES)], axis=1)   # [chunk, core, 64, C]
    return np.ascontiguousarray(arr.reshape(B, T, C))
